# revision 22
# baseline (speedup 1.0000x reference)
"""Trainium2 Bass kernel for DietConv2dV2: 3x3 conv (stride 1, pad 1) + bias.

x: (16, 8, 1024, 1024) fp32, weight: (8, 8, 3, 3), bias: (8,) -> out like x.

Strategy
--------
Data-parallel: 16 images / 8 cores = 2 images per core, no collectives.

Per core the conv runs as a banded matmul on the PE array:
  - K (contraction, partitions) = 16 input rows x 8 in-channels = 128,
    partition p = hi*8 + ci.
  - M (stationary free dim)     = 14 out rows x 8 out-channels = 112,
    column  m = parity*56 + co*7 + r  where ho = 2r + parity.
  - N (moving free dim)         = 512-wide w chunk.
The stationary "band" matrix covers all 3 kh taps at once; the 3 kw taps
are 3 PSUM-accumulated matmuls reading the same SBUF rows at shifted w
offsets.  Three band variants (first/mid/last) absorb the h-edge padding
into the weights, so every block is a full 16-row load with no row
memsets.  Band matrices are precomputed on the host from `weight`.

I/O runs in fp16 (host-cast both ways): halves HBM traffic vs fp32 for
~1e-3 rel err.  The DMA subsystem is packet-rate-bound, so layouts are
chosen to make 4KB descriptors:
  - input: SBUF write offset 4B-aligned (2 pad cols), SWDGE aggregates
    the 2KB row descriptors into 4KB packets;
  - output: parity-paired ot tile [56, 2048] gives 4KB SBUF lines that
    land on 4KB-contiguous HBM (rows 2r, 2r+1 adjacent), so HWDGE
    descriptors are 4KB natively.
The PSUM->SBUF bias eviction is split across DVE / Pool / Activation so
no single engine binds; output DMAs ride the sync HWDGE ring, input the
SWDGE ring (Pool-triggered).
"""

import numpy as np

import bass_rust
import concourse.bass as bass
import concourse.mybir as mybir
from concourse.tile import TileContext
from concourse.bass_utils import run_bass_kernel_spmd

F32 = mybir.dt.float32
F16 = mybir.dt.float16

N_CORES = 8
IMG_PER_CORE = 2
C = 8          # channels (in == out)
H = 1024
W = 1024
KS = 3         # kernel size
HB = 14        # output rows per block (16 input rows -> 14 output rows)
KROWS = HB + KS - 1  # 16 input rows per block
M = C * HB     # 112 live stationary columns
MH = M // 2    # 56: one parity's worth
MP = 128       # padded stationary width: parity0 at 0..55, parity1 at
               # 64..119 (Activation reads need 32-aligned partition
               # starts), dummy zero columns elsewhere
P1 = 64        # parity1 partition base
WCHUNK = 512   # PSUM bank = 512 fp32


def _split_excess_waits(nc):
    """This walrus build accepts 1 sync-wait per instruction (2 for
    EventSemaphore); Tile's final drain and ldweights can end up with
    more.  Move overflow waits onto EventSemaphore carriers inserted
    before the offender on the same engine."""
    for fn in nc.m.functions:
        for blk in fn.blocks:
            out = []
            changed = False
            for inst in blk.instructions:
                si = inst.sync_info
                cap = 2 if inst.opcode == "EventSemaphore" else 1
                waits = list(si.on_wait) if si is not None else []
                if len(waits) > cap:
                    changed = True
                    overflow, keep = waits[:-cap], waits[-cap:]
                    for j in range(0, len(overflow), 2):
                        es = mybir.InstEventSemaphore(
                            name=nc.get_next_instruction_name(), ins=[], outs=[]
                        )
                        es.engine = inst.engine
                        es.sync_info = bass_rust.SyncInfo(
                            on_wait=overflow[j : j + 2], on_update=[]
                        )
                        nc.register_instruction(es, overwrite=True)
                        out.append(es)
                    inst.sync_info = bass_rust.SyncInfo(
                        on_wait=keep, on_update=list(si.on_update)
                    )
                out.append(inst)
            if changed:
                blk.instructions = out


def _build(nimg, h, w, reps=1, salt=0):
    assert h % 2 == 0 and (h - 2) % HB == 0, "blocking needs h = 14k + 2, even"
    nblocks = (h - 2) // HB + 1  # first + middles + last (1024 -> 74)

    nc = bass.Bass(name=f"dietconv_s{salt}")
    x = nc.dram_tensor("x", [nimg, C, h, w], F16, kind="ExternalInput")
    wb = nc.dram_tensor("wband", [3, KS, 128, MP], F16, kind="ExternalInput")
    bv = nc.dram_tensor("biasv", [MP, 1], F32, kind="ExternalInput")
    out = nc.dram_tensor("out", [nimg, C, h, w], F16, kind="ExternalOutput")

    # row-major (h, c) view so SBUF partition p = hi*8 + ci
    xr = x.rearrange("n c h w -> n h c w")
    # (row-pair, 2w) view: block h0 = 14b is always even, so its 7
    # row-pairs are a clean slice [q0, q0+7)
    outv = out.rearrange("n c (q p) w -> n c q (p w)", p=2)

    with TileContext(nc) as tc:
        with (
            tc.tile_pool(name="wpool", bufs=1) as wpool,
            tc.tile_pool(name="xpool", bufs=8) as xpool,
            tc.tile_pool(name="opool", bufs=6) as opool,
            tc.tile_pool(name="pspool", bufs=4, space="PSUM") as pspool,
        ):
            wts = []
            for v in range(3):
                row = []
                for kw in range(KS):
                    wt = wpool.tile([128, MP], F16, name=f"wt{v}_{kw}")
                    nc.gpsimd.dma_start(out=wt[:], in_=wb[v, kw])
                    row.append(wt)
                wts.append(row)
            bt = wpool.tile([MP, 1], F32, name="bt")
            nc.sync.dma_start(out=bt[:], in_=bv[:])

            # fixed ring of input tiles whose pad columns (0-1 and
            # w+2..w+3) are zeroed exactly once -- the per-block DMA only
            # touches cols 2..w+2, so the pads stay zero across reuse
            NXT = 8
            xts = []
            for i in range(NXT):
                xt = xpool.tile([128, w + 4], F16, name=f"xt{i}")
                nc.vector.memset(xt[:, 0:2], 0.0)
                nc.vector.memset(xt[:, w + 2 : w + 4], 0.0)
                xts.append(xt)

            def body():
                blkno = 0
                for n in range(nimg):
                    for b in range(nblocks):
                        if b == 0:
                            r0, h0, v = 0, 0, 0
                        elif b < nblocks - 1:
                            r0, h0, v = HB * b - 1, HB * b, 1
                        else:
                            # last block recomputes 12 rows already
                            # written by block nblocks-2 (identical
                            # values) to stay shape-regular
                            r0, h0, v = h - KROWS, h - HB, 2
                        # tile col c holds input w = c-2; cols 0-1 and
                        # w+2..w+3 are zero pad (4B-aligned DMA offset)
                        xt = xts[blkno % NXT]
                        blkno += 1
                        nc.gpsimd.dma_start(
                            out=xt[:, 2 : w + 2], in_=xr[n, r0 : r0 + KROWS, :, :]
                        )
                        # one PSUM tile spanning both w-chunks (2 banks);
                        # each matmul stays within one bank
                        ps = pspool.tile([MP, w], F32, name="ps", tag="ps")
                        for j in range(w // WCHUNK):
                            base = j * WCHUNK
                            for kw in range(KS):
                                c0 = base + kw + 1
                                nc.tensor.matmul(
                                    ps[:, base : base + WCHUNK],
                                    wts[v][kw][:],
                                    xt[:, c0 : c0 + WCHUNK],
                                    start=(kw == 0),
                                    stop=(kw == KS - 1),
                                )
                        # parity-paired eviction: ot line = [row 2r | row
                        # 2r+1] (4KB) so output DMA descriptors are 4KB.
                        # Split along free dim across DVE/Pool/Act (lane-
                        # parallel engines; only column splits cut time).
                        ot = opool.tile([MH, 2 * w], F16, name="ot", tag="ot")
                        nc.vector.tensor_scalar_add(
                            ot[:, 0:w], ps[0:MH, :], bt[0:MH]
                        )
                        nc.scalar.activation(
                            ot[:, w : 2 * w],
                            ps[P1 : P1 + MH, :],
                            mybir.ActivationFunctionType.Identity,
                            bias=bt[P1 : P1 + MH],
                        )
                        nc.sync.dma_start(
                            out=outv[n, :, h0 // 2 : h0 // 2 + MH // C, :],
                            in_=ot[:],
                        )

            # static unroll: tc.For_i loop control hits a walrus codegen
            # gap in this build ("ISA wrong length" on CompareAndBranch)
            for _ in range(reps):
                body()

    _split_excess_waits(nc)
    return nc


def _band_inputs(weight, bias):
    """Band matrices for m = parity*56 + co*7 + r (ho = 2r + parity).

    Variant v in {0: first, 1: mid, 2: last} maps tap kh to input row
    hi = ho + (v - 1) + kh; taps falling outside [0, 16) are dropped
    (they correspond to the conv's zero padding)."""
    weight = np.asarray(weight, dtype=np.float32)
    bias = np.asarray(bias, dtype=np.float32)
    S = np.zeros((3, KS, 128, MP), dtype=np.float16)
    for v in range(3):
        for kw in range(KS):
            for ho in range(HB):
                parity, r = ho % 2, ho // 2
                m0 = parity * P1 + r
                for kh in range(KS):
                    hi = ho + (v - 1) + kh
                    if not 0 <= hi < KROWS:
                        continue
                    blk = weight[:, :, kh, kw].T.astype(np.float16)  # [ci, co]
                    S[v, kw, hi * C : (hi + 1) * C, m0 : m0 + MH : HB // 2] = blk
    biasv = np.zeros((MP, 1), dtype=np.float32)
    rep = np.repeat(bias, HB // 2)  # [56] = bias[co] at co*7 + r
    biasv[0:MH, 0] = rep
    biasv[P1 : P1 + MH, 0] = rep
    return S, biasv


def _run(x, weight, bias, nimg_per_core, h, w, n_cores, reps=1):
    S, biasv = _band_inputs(weight, bias)
    x = np.ascontiguousarray(np.asarray(x).astype(np.float16))
    in_maps = [
        {
            "x": x[i * nimg_per_core : (i + 1) * nimg_per_core],
            "wband": S,
            "biasv": biasv,
        }
        for i in range(n_cores)
    ]
    # The walrus backend compile is rarely flaky (parallel codegen race).
    # jax caches the failed compilation by HLO, so retries must change the
    # BIR bytes (salt) and drop the jit cache.
    last_exc = None
    for attempt in range(4):
        try:
            nc = _build(nimg_per_core, h, w, reps, salt=attempt)
            res = run_bass_kernel_spmd(nc, in_maps, core_ids=list(range(n_cores)))
            break
        except Exception as e:  # noqa: BLE001
            last_exc = e
            try:
                import jax

                jax.clear_caches()
            except Exception:  # noqa: BLE001
                pass
    else:
        raise last_exc
    return np.concatenate(
        [np.asarray(r["out"]).astype(np.float32) for r in res.results], axis=0
    )


def kernel(x, weight, bias):
    return _run(x, weight, bias, IMG_PER_CORE, H, W, N_CORES, reps=1)
